# revision 34
# baseline (speedup 1.0000x reference)
"""Bidirectional Mamba block (in_proj -> depthwise causal conv -> SiLU ->
forward+backward S6 selective scan -> gated combine -> out_proj) as a
Trainium2 Bass/Tile SPMD kernel over 8 NeuronCores.

Sharding: tensor-parallel over d_inner (256 channels per core). The conv and
the S6 scans are channel-independent, so they need no communication. Two
small collectives:
  * AllReduce (bf16) of the partial x-projection dbc = u @ Wx^T per direction
  * Chunked ReduceScatter of the partial out-projection, overlapped with the
    out_proj matmuls; the host reassembles the 8 shards.

Compute dtypes: bf16 operands everywhere (fp32 PSUM accumulation), which
doubles/quadruples DVE elementwise throughput and halves DMA traffic. The S6
recurrence runs on the DVE tensor_tensor_scan (fp32 internal state).
Activation-table usage is phase-ordered (Silu early, Exp/Ln for the scan
phase) to avoid ACT_TABLE_LOAD thrash.
"""

import os
import sys

for _p in ("/opt/trn_rl_repo", "/root/.axon_site/_ro/trn_rl_repo"):
    if os.path.isdir(_p) and _p not in sys.path:
        sys.path.append(_p)

from dataclasses import dataclass

import ml_dtypes
import numpy as np

import concourse.bass as bass
import concourse.mybir as mybir
import concourse.tile as tile
from concourse import bacc

DT = mybir.dt.float32
BF = mybir.dt.bfloat16
AF = mybir.ActivationFunctionType
OP = mybir.AluOpType


@dataclass(frozen=True)
class Cfg:
    n_cores: int = 8
    B: int = 2
    L: int = 1024
    M: int = 1024      # d_model
    DI: int = 2048     # d_inner
    N: int = 16        # d_state
    R: int = 64        # dt_rank
    KC: int = 4        # conv kernel
    RSC: int = 2       # ReduceScatter chunks

    @property
    def DC(self):  # channels per core
        return self.DI // self.n_cores

    @property
    def TOK(self):
        return self.B * self.L

    @property
    def P_CH(self):  # partitions per channel tile
        return min(128, self.DC)

    @property
    def CHT(self):  # channel tiles per core
        return self.DC // self.P_CH

    @property
    def NT(self):  # scan tiles per (dir, batch): 8 channels each
        return self.DC // 8

    @property
    def TPC(self):  # scan tiles per channel tile
        return self.P_CH // 8

    @property
    def FCH(self):  # matmul moving-dim chunk over tokens (never spans batches)
        return min(512, self.L)

    @property
    def E(self):
        return self.R + 2 * self.N

    def check(self):
        assert self.DC % 8 == 0 and self.DC % self.P_CH == 0
        assert self.M % 128 == 0
        assert self.TOK % 128 == 0 and self.TOK % self.FCH == 0
        assert self.L % min(512, self.L) == 0
        assert self.N == 16
        assert self.TOK % (self.RSC * self.n_cores) == 0


FULL = Cfg()


def build_consts(cfg: Cfg):
    """Selection matrices used as PE 'weights' (exact 0/1 values)."""
    P = 128
    ident = np.eye(P, dtype=np.float32)
    # R_all[:, jj, :]: out[p] = src[8*jj + p//16]  (delta/w replication)
    r_all = np.zeros((cfg.P_CH, cfg.TPC, P), np.float32)
    for jj in range(cfg.TPC):
        for p in range(P):
            r_all[8 * jj + p // 16, jj, p] = 1.0
    # T_sel[:, which, :]: out[p] = src[16*which + p%16]  (B/C replication)
    t_sel = np.zeros((2 * cfg.N, 2, P), np.float32)
    for which in range(2):
        for p in range(P):
            t_sel[cfg.N * which + p % 16, which, p] = 1.0
    # S_all[:, jj, :]: reduce groups of 16 partitions into channel 8*jj+p//16
    s_all = np.zeros((P, cfg.TPC, cfg.P_CH), np.float32)
    for jj in range(cfg.TPC):
        for p in range(P):
            s_all[p, jj, 8 * jj + p // 16] = 1.0
    return ident, r_all, t_sel, s_all


def build_program(cfg: Cfg) -> bass.Bass:
    cfg.check()
    P = 128
    TOK, L, M = cfg.TOK, cfg.L, cfg.M
    DC, CHT, P_CH, NT, TPC, FCH = (cfg.DC, cfg.CHT, cfg.P_CH, cfg.NT,
                                   cfg.TPC, cfg.FCH)
    MT = M // P               # m tiles
    TBT = TOK // P            # token blocks
    NFC = TOK // FCH          # token chunks
    E, R, N = cfg.E, cfg.R, cfg.N
    LH = min(512, L)          # matmul chunk within one sequence
    NLH = L // LH

    nc = bacc.Bacc(
        "TRN2", target_bir_lowering=False, debug=False, num_devices=cfg.n_cores
    )

    # ---- kernel I/O ----
    x_d = nc.dram_tensor("x", [M, TOK], BF, kind="ExternalInput")
    winuT_d = nc.dram_tensor("winuT", [M, DC], BF, kind="ExternalInput")
    winrT_d = nc.dram_tensor("winrT", [M, DC], BF, kind="ExternalInput")
    wconv_d = nc.dram_tensor("wconv", [P, CHT * cfg.KC], DT, kind="ExternalInput")
    bconv_d = nc.dram_tensor("bconv", [P, CHT], DT, kind="ExternalInput")
    wxT_d = {d: nc.dram_tensor(f"wx{d}T", [DC, E], BF, kind="ExternalInput")
             for d in "fb"}
    wdtT_d = {d: nc.dram_tensor(f"wdt{d}T", [R, DC], BF, kind="ExternalInput")
              for d in "fb"}
    bdt_d = {d: nc.dram_tensor(f"bdt{d}", [P, CHT], DT, kind="ExternalInput")
             for d in "fb"}
    acol_d = {d: nc.dram_tensor(f"acol{d}", [P_CH, CHT * N], DT,
                                kind="ExternalInput")
              for d in "fb"}
    dsum_d = nc.dram_tensor("dsum", [P, CHT], DT, kind="ExternalInput")
    woutT_d = nc.dram_tensor("woutT", [DC, M], BF, kind="ExternalInput")
    ident_d = nc.dram_tensor("ident", [P, P], BF, kind="ExternalInput")

    out_d = nc.dram_tensor("out_rs", [TOK // cfg.n_cores, M], BF,
                           kind="ExternalOutput")
    rg = [list(range(cfg.n_cores))]
    cc_space = "Shared" if cfg.n_cores > 4 else "Local"

    with tile.TileContext(nc) as tc:
        with tc.tile_pool(name="persist", bufs=1) as pp, \
             tc.tile_pool(name="dram", bufs=1, space="DRAM") as dp:

            # ---------- persistent SBUF (small weights + gate activations) --
            ident_s = pp.tile([P, P], BF)
            nc.sync.dma_start(ident_s[:], ident_d.ap())
            wconv_s = pp.tile([P, CHT, cfg.KC], DT)
            nc.sync.dma_start(wconv_s[:], wconv_d.ap().rearrange(
                "p (c k) -> p c k", c=CHT))
            bconv_s = pp.tile([P, CHT], DT)
            nc.sync.dma_start(bconv_s[:], bconv_d.ap())
            wx_s, wdt_s, bdt_s, acol_s = {}, {}, {}, {}
            for d in "fb":
                wx_s[d] = pp.tile([P_CH, CHT, E], BF, name=f"wx{d}_s")
                nc.sync.dma_start(wx_s[d][:], wxT_d[d].ap().rearrange(
                    "(c p) e -> p c e", p=P_CH))
                wdt_s[d] = pp.tile([R, DC], BF, name=f"wdt{d}_s")
                nc.sync.dma_start(wdt_s[d][:], wdtT_d[d].ap())
                bdt_s[d] = pp.tile([P, CHT], DT, name=f"bdt{d}_s")
                nc.sync.dma_start(bdt_s[d][:], bdt_d[d].ap())
                acol_s[d] = pp.tile([P_CH, CHT, N], DT, name=f"acol{d}_s")
                nc.sync.dma_start(acol_s[d][:], acol_d[d].ap().rearrange(
                    "p (c n) -> p c n", c=CHT))
            dsum_s = pp.tile([P, CHT], DT)
            nc.sync.dma_start(dsum_s[:], dsum_d.ap())
            wout_s = pp.tile([P_CH, CHT, M], BF)
            nc.sync.dma_start(wout_s[:], woutT_d.ap().rearrange(
                "(c p) m -> p c m", p=P_CH))

            u_c = [pp.tile([P_CH, TOK], BF, name=f"u_c{c}") for c in range(CHT)]
            sres = [pp.tile([P_CH, TOK], BF, name=f"sres{c}")
                    for c in range(CHT)]
            uD = [pp.tile([P_CH, TOK], BF, name=f"uD{c}") for c in range(CHT)]

            # ---------- phase 0-2: x^T, in_proj, conv, silu ----------
            with tc.tile_pool(name="proj", bufs=1) as jp, \
                 tc.tile_pool(name="proj_ps", bufs=1, space="PSUM") as jpp:
                xT = [jp.tile([P, TOK], BF, name=f"xT{mt}") for mt in range(MT)]
                win_s = jp.tile([P, MT, 2 * DC], BF)
                nc.sync.dma_start(win_s[:, :, :DC], winuT_d.ap().rearrange(
                    "(a p) c -> p a c", p=P))
                nc.sync.dma_start(win_s[:, :, DC:], winrT_d.ap().rearrange(
                    "(a p) c -> p a c", p=P))

                # x arrives pre-transposed [M, TOK] from the host: load
                # the xT tiles directly (no PE transposes / evac copies)
                for mt in range(MT):
                    nc.sync.dma_start(xT[mt][:],
                                      x_d.ap()[mt * P:(mt + 1) * P, :])

                # padded conv inputs (filled by in_proj PSUM evacuation)
                upad = [[jp.tile([P_CH, cfg.KC - 1 + L], BF,
                                 name=f"upad{c}_{b}")
                         for b in range(cfg.B)] for c in range(CHT)]
                for c in range(CHT):
                    for b in range(cfg.B):
                        nc.gpsimd.memset(upad[c][b][:, :cfg.KC - 1], 0.0)

                # ------ phase 3: dbc partials (bf16, both directions),
                # one AllReduce PER BATCH so batch 0's collective hides under
                # batch 1's conv/dbc and batch 1's under batch 0's scans ----
                dbc_part = [dp.tile([2 * E, L], BF, name=f"dbc_part{b}")
                            for b in range(cfg.B)]
                dbc_red = [dp.tile([2 * E, L], BF, addr_space=cc_space,
                                   name=f"dbc_red{b}") for b in range(cfg.B)]
                OFF = {"f": 0, "b": E}

                def dbc_batch(b):
                    for fc in range(b * (L // FCH), (b + 1) * (L // FCH)):
                        f0 = fc * FCH
                        for d in "fb":
                            bps = jpp.tile([E, FCH], DT, tag="mm", bufs=4,
                                           name="bps")
                            for c in range(CHT):
                                nc.tensor.matmul(
                                    bps[:],
                                    wx_s[d][:, c, :],
                                    u_c[c][:, f0:f0 + FCH],
                                    start=(c == 0), stop=(c == CHT - 1))
                            bst = jp.tile([E, FCH], BF, tag="bst", bufs=3,
                                          name="bst")
                            nc.scalar.copy(bst[:], bps[:])
                            nc.sync.dma_start(
                                dbc_part[b][OFF[d]:OFF[d] + E,
                                            f0 - b * L:f0 - b * L + FCH],
                                bst[:])
                    nc.gpsimd.collective_compute(
                        "AllReduce", OP.add, replica_groups=rg,
                        ins=[dbc_part[b].opt()], outs=[dbc_red[b].opt()])

                with tc.tile_pool(name="conv", bufs=1) as cp:
                    for b in range(cfg.B):
                        # in_proj chunks of this batch
                        for fc in range(b * (L // FCH),
                                        (b + 1) * (L // FCH)):
                            for c in range(CHT):
                                f0 = fc * FCH
                                ups = jpp.tile([P_CH, FCH], DT, tag="mm",
                                               bufs=4, name="ups")
                                for kt in range(MT):
                                    nc.tensor.matmul(
                                        ups[:],
                                        win_s[:, kt, c * P_CH:(c + 1) * P_CH],
                                        xT[kt][:, f0:f0 + FCH],
                                        start=(kt == 0), stop=(kt == MT - 1))
                                off = f0 % L
                                nc.scalar.copy(
                                    upad[c][b][:, cfg.KC - 1 + off:
                                               cfg.KC - 1 + off + FCH],
                                    ups[:])
                        # depthwise causal conv (tap products + tree add,
                        # all bf16 4x tensor_scalar/tensor_tensor) + SiLU
                        for c in range(CHT):
                            tp_ = []
                            for k in range(cfg.KC):
                                tap = upad[c][b][:, k:k + L]
                                wk = wconv_s[:P_CH, c, k:k + 1]
                                t_ = cp.tile([P_CH, L], BF, tag=f"ct{k}",
                                             bufs=2, name=f"ct{k}")
                                if k == 0:
                                    nc.vector.tensor_scalar(
                                        t_[:], tap, wk,
                                        bconv_s[:P_CH, c:c + 1],
                                        OP.mult, OP.add)
                                else:
                                    nc.vector.tensor_scalar(
                                        t_[:], tap, wk, None, OP.mult)
                                tp_.append(t_)
                            s01 = cp.tile([P_CH, L], BF, tag="s01", bufs=2,
                                          name="s01")
                            nc.vector.tensor_tensor(s01[:], tp_[0][:],
                                                    tp_[1][:], OP.add)
                            s23 = cp.tile([P_CH, L], BF, tag="s23", bufs=2,
                                          name="s23")
                            nc.vector.tensor_tensor(s23[:], tp_[2][:],
                                                    tp_[3][:], OP.add)
                            acc = cp.tile([P_CH, L], BF, tag="cacc", bufs=2,
                                          name="cacc")
                            nc.vector.tensor_tensor(acc[:], s01[:], s23[:],
                                                    OP.add)
                            nc.scalar.activation(
                                u_c[c][:, b * L:(b + 1) * L], acc[:],
                                AF.Silu)
                            nc.vector.tensor_scalar(
                                uD[c][:, b * L:(b + 1) * L],
                                u_c[c][:, b * L:(b + 1) * L],
                                dsum_s[:P_CH, c:c + 1], None, OP.mult)
                        dbc_batch(b)
                # res projection overlaps the AllReduce flights
                for c in range(CHT):
                    for fc in range(NFC):
                        f0 = fc * FCH
                        rps = jpp.tile([P_CH, FCH], DT, tag="mm", bufs=4,
                                       name="rps")
                        for kt in range(MT):
                            nc.tensor.matmul(
                                rps[:],
                                win_s[:, kt, DC + c * P_CH:DC + (c + 1) * P_CH],
                                xT[kt][:, f0:f0 + FCH],
                                start=(kt == 0), stop=(kt == MT - 1))
                        # sres = silu(res); the 0.5 factor is folded into
                        # W_out host-side
                        nc.scalar.activation(sres[c][:, f0:f0 + FCH], rps[:],
                                             AF.Silu)

            # ---------- phase 4+5: per-batch delta prep, scans, out_proj
            # with one ReduceScatter per batch ----------
            # Channel-partition layout: each scan tile is [128 channels,
            # G states x L tokens] with the state index in the FREE dim.
            # B/C rows are broadcast across partitions straight from the
            # per-batch AllReduce result in DRAM, so dA/dBu/hC are pure SBUF
            # bf16 elementwise ops and the only PE work in the scan loop is
            # the identity-accumulate that sums hC over states.  dA is zeroed
            # at every state-segment's first element (in scan order).
            # Batch 0's out_proj + ReduceScatter overlap batch 1's scans.
            y_f = [pp.tile([P_CH, TOK], BF, name=f"y_f{c}") for c in range(CHT)]
            G = 2                      # states per scan group
            NG = N // G                # groups per (dir, channel tile)
            TBB = L // P               # token blocks per batch
            RSH = L // cfg.n_cores     # rows per core per batch RS
            MFC = min(512, M)
            out_part = [dp.tile([L, M], BF, name=f"out_part{b}")
                        for b in range(cfg.B)]
            out_rs = [dp.tile([RSH, M], BF, name=f"out_rs{b}")
                      for b in range(cfg.B)]

            with tc.tile_pool(name="scan_sb", bufs=1) as sp, \
                 tc.tile_pool(name="scan_ps", bufs=1, space="PSUM") as spp, \
                 tc.tile_pool(name="comb", bufs=1) as kp:
                for b in range(cfg.B):
                    bl = slice(b * L, (b + 1) * L)
                    # delta/w2 prep per direction; the backward prep is
                    # deferred until after the first forward scan group so it
                    # doesn't gate the first dA exps in the ACT queue
                    delta, w2 = {}, {}

                    def prep_dir(d):
                        dt_sb = sp.tile([R, L], BF, tag=f"dt{d}", bufs=2,
                                        name=f"dt_{d}")
                        nc.sync.dma_start(dt_sb[:],
                                          dbc_red[b][OFF[d]:OFF[d] + R, :])
                        delta[d] = [sp.tile([P_CH, L], BF,
                                            tag=f"delta{d}{c}", bufs=2,
                                            name=f"delta_{d}{c}")
                                    for c in range(CHT)]
                        w2[d] = [sp.tile([P_CH, G, L], BF, tag=f"w2{d}{c}",
                                         bufs=2, name=f"w2_{d}{c}")
                                 for c in range(CHT)]
                        spt = [sp.tile([P_CH, L], BF, tag=f"spt{c}", bufs=2,
                                       name=f"spt{c}") for c in range(CHT)]
                        for c in range(CHT):
                            for fc in range(L // FCH):
                                f0 = fc * FCH
                                dps = spp.tile([P_CH, FCH], DT, tag="rep",
                                               bufs=2, name="dps")
                                nc.tensor.matmul(
                                    dps[:],
                                    wdt_s[d][:, c * P_CH:(c + 1) * P_CH],
                                    dt_sb[:, f0:f0 + FCH],
                                    start=True, stop=True)
                                # softplus(x + bdt) = ln(1 + exp(x + bdt))
                                nc.scalar.activation(
                                    spt[c][:, f0:f0 + FCH], dps[:], AF.Exp,
                                    bias=bdt_s[d][:P_CH, c:c + 1])
                        for c in range(CHT):
                            nc.scalar.activation(delta[d][c][:], spt[c][:],
                                                 AF.Ln, bias=1.0)
                            nc.vector.tensor_tensor(
                                w2[d][c][:, 0, :], delta[d][c][:],
                                u_c[c][:, bl], OP.mult)
                            for i in range(1, G):
                                nc.sync.dma_start(w2[d][c][:, i, :],
                                                  w2[d][c][:, 0, :])

                    def scan_cd(c, d):
                            y_ps = spp.tile([P_CH, L], DT, tag="y", bufs=2,
                                            name="y_ps")
                            for g in range(NG):
                                n0 = g * G
                                o_b = OFF[d] + R + n0
                                Bg = sp.tile([P, G, L], BF, tag="Bg", bufs=4,
                                             name="Bg")
                                nc.sync.dma_start(
                                    Bg[:],
                                    dbc_red[b][o_b:o_b + G, :]
                                    .unsqueeze(0).broadcast_to([P, G, L]))
                                Cg = sp.tile([P, G, L], BF, tag="Cg", bufs=4,
                                             name="Cg")
                                nc.sync.dma_start(
                                    Cg[:],
                                    dbc_red[b][o_b + N:o_b + N + G, :]
                                    .unsqueeze(0).broadcast_to([P, G, L]))

                                dA = sp.tile([P_CH, G, L], BF, tag="dA",
                                             bufs=4, name="dA")
                                for i in range(G):
                                    nc.scalar.activation(
                                        dA[:, i, :], delta[d][c][:], AF.Exp,
                                        scale=acol_s[d][:P_CH, c,
                                                        n0 + i:n0 + i + 1])
                                dBu = sp.tile([P_CH, G, L], BF, tag="dBu",
                                              bufs=3, name="dBu")
                                nc.vector.tensor_tensor(
                                    dBu[:].rearrange("p a b -> p (a b)"),
                                    w2[d][c][:].rearrange("p a b -> p (a b)"),
                                    Bg[:].rearrange("p a b -> p (a b)"),
                                    OP.mult)
                                # zero dA at every state-segment start (scan
                                # order); the very first element is harmless
                                # because the scan initial is 0.
                                flat = dA[:].rearrange("p a b -> p (a b)")
                                if d == "f":
                                    nc.vector.memset(flat[:, 0::L], 0.0)
                                else:
                                    nc.vector.memset(flat[:, L - 1::L], 0.0)
                                h = sp.tile([P_CH, G, L], BF, tag="h",
                                            bufs=3, name="h")
                                hf = h[:].rearrange("p a b -> p (a b)")
                                dAf = dA[:].rearrange("p a b -> p (a b)")
                                dBuf = dBu[:].rearrange("p a b -> p (a b)")
                                if d == "f":
                                    nc.vector.tensor_tensor_scan(
                                        hf, dAf, dBuf, 0.0, OP.mult, OP.add)
                                else:
                                    nc.vector.tensor_tensor_scan(
                                        hf[:, ::-1], dAf[:, ::-1],
                                        dBuf[:, ::-1], 0.0, OP.mult, OP.add)
                                # hC in place: h *= Cg
                                nc.vector.tensor_tensor(
                                    hf, hf,
                                    Cg[:].rearrange("p a b -> p (a b)"),
                                    OP.mult)
                                # y += hC summed over the G states
                                for i in range(G):
                                    for lh in range(L // LH):
                                        q = lh * LH
                                        nc.tensor.matmul(
                                            y_ps[:, q:q + LH],
                                            ident_s[:],
                                            h[:, i, q:q + LH],
                                            start=(g == 0 and i == 0),
                                            stop=(g == NG - 1 and i == G - 1))
                            # evacuate / combine (all-SBUF bf16 ops)
                            if d == "f":
                                nc.scalar.copy(y_f[c][:, bl], y_ps[:])
                            else:
                                # y = (y_f + y_b + u*(fD+bD)) * silu(res)
                                # (the 0.5 is folded into W_out host-side)
                                yb = kp.tile([P_CH, L], BF, tag="t5", bufs=2,
                                             name="yb")
                                nc.scalar.copy(yb[:], y_ps[:])
                                t1 = kp.tile([P_CH, L], BF, tag="t5", bufs=2,
                                             name="t1")
                                nc.vector.tensor_tensor(t1[:], yb[:],
                                                        y_f[c][:, bl], OP.add)
                                t2 = kp.tile([P_CH, L], BF, tag="t5", bufs=2,
                                             name="t2")
                                nc.vector.tensor_tensor(t2[:], t1[:],
                                                        uD[c][:, bl], OP.add)
                                nc.vector.tensor_tensor(
                                    y_f[c][:, bl], t2[:], sres[c][:, bl],
                                    OP.mult)

                    prep_dir("f")
                    scan_cd(0, "f")
                    prep_dir("b")
                    scan_cd(0, "b")
                    scan_cd(1, "f")
                    scan_cd(1, "b")

                    # ---- out_proj for this batch (both channel tiles
                    # accumulated in PSUM) + one ReduceScatter ----
                    for tbl in range(TBB):
                        tb = b * TBB + tbl
                        ost = kp.tile([P, M], BF, tag="ost", bufs=2,
                                      name="ost")
                        for mc in range(M // MFC):
                            o = mc * MFC
                            ops = spp.tile([P, MFC], DT, tag="out", bufs=2,
                                           name="ops")
                            for c in range(CHT):
                                nc.tensor.matmul(
                                    ops[:],
                                    y_f[c][:, tb * P:(tb + 1) * P],
                                    wout_s[:, c, o:o + MFC],
                                    start=(c == 0), stop=(c == CHT - 1))
                            nc.scalar.copy(ost[:, o:o + MFC], ops[:])
                        nc.sync.dma_start(
                            out_part[b][tbl * P:(tbl + 1) * P, :], ost[:])
                    nc.gpsimd.collective_compute(
                        "ReduceScatter", OP.add, replica_groups=rg,
                        ins=[out_part[b].opt()], outs=[out_rs[b].opt()])
                    nc.sync.dma_start(
                        out_d.ap()[b * RSH:(b + 1) * RSH, :], out_rs[b][:])

    nc.compile()
    return nc


# --------------------------------------------------------------------------
# host side
# --------------------------------------------------------------------------

def host_prep(cfg: Cfg, inputs: dict) -> list[dict]:
    """Slice the full-model inputs into one input map per core."""
    P = 128
    f32 = np.float32
    bf16 = ml_dtypes.bfloat16

    def g(name):
        return np.asarray(inputs[name], f32)

    x = g("x").reshape(cfg.TOK, cfg.M)
    W_in = g("W_in")
    W_conv = g("W_conv").reshape(cfg.DI, cfg.KC)
    b_conv = g("b_conv")
    W_out = g("W_out")
    ident = np.eye(P, dtype=np.float32)

    per = {}
    for d in "fb":
        per[d] = dict(
            A=-np.exp(g(d + "A_log")),            # (DI, N)
            D=g(d + "D"),
            Wx=g(d + "Wx"),                       # (E, DI)
            Wdt=g(d + "Wdt"),                     # (DI, R)
            bdt=g(d + "bdt"),
        )

    def col_layout(v):  # (DC,) -> (P_CH, CHT): [p, c] = v[c*P_CH + p]
        return np.ascontiguousarray(
            v.reshape(cfg.CHT, cfg.P_CH).T.astype(f32))

    def pad_p(a):  # pad partition dim up to 128
        if a.shape[0] == P:
            return np.ascontiguousarray(a.astype(f32))
        out = np.zeros((P,) + a.shape[1:], f32)
        out[:a.shape[0]] = a
        return out

    in_maps = []
    for core in range(cfg.n_cores):
        c0 = core * cfg.DC
        ch = slice(c0, c0 + cfg.DC)
        m = {
            "x": np.ascontiguousarray(x.T).astype(bf16),
            "winuT": np.ascontiguousarray(W_in[ch, :].T).astype(bf16),
            "winrT": np.ascontiguousarray(
                W_in[cfg.DI + c0:cfg.DI + c0 + cfg.DC, :].T).astype(bf16),
            "wconv": pad_p(
                W_conv[ch].reshape(cfg.CHT, cfg.P_CH, cfg.KC)
                .transpose(1, 0, 2).reshape(cfg.P_CH, cfg.CHT * cfg.KC)),
            "bconv": pad_p(col_layout(b_conv[ch])),
            "dsum": pad_p(col_layout(per["f"]["D"][ch] + per["b"]["D"][ch])),
            "woutT": np.ascontiguousarray(W_out[:, ch].T * 0.5).astype(bf16),
            "ident": ident.astype(bf16),
        }
        for d in "fb":
            pd = per[d]
            m[f"wx{d}T"] = np.ascontiguousarray(pd["Wx"][:, ch].T).astype(bf16)
            m[f"wdt{d}T"] = np.ascontiguousarray(pd["Wdt"][ch, :].T).astype(bf16)
            m[f"bdt{d}"] = pad_p(col_layout(pd["bdt"][ch]))
            # A columns: [p, (c, n)] = A[c*P_CH + p, n] (local channels)
            Ac = pd["A"][ch]                       # (DC, N)
            m[f"acol{d}"] = np.ascontiguousarray(
                Ac.reshape(cfg.CHT, cfg.P_CH, cfg.N)
                .transpose(1, 0, 2).reshape(cfg.P_CH, cfg.CHT * cfg.N)
                .astype(f32))
        in_maps.append({k: np.ascontiguousarray(v) for k, v in m.items()})
    return in_maps


def gather_out(cfg: Cfg, results: list[dict]) -> np.ndarray:
    """Reassemble chunked-ReduceScatter shards.

    Core c's out_rs rows [k*RSH, (k+1)*RSH) correspond to global token rows
    [k*RCH + c*RSH, k*RCH + (c+1)*RSH).
    """
    RCH = cfg.TOK // cfg.RSC
    RSH = RCH // cfg.n_cores
    out = np.empty((cfg.TOK, cfg.M), np.float32)
    for c in range(cfg.n_cores):
        shard = np.asarray(results[c]["out_rs"]).astype(np.float32)
        for k in range(cfg.RSC):
            out[k * RCH + c * RSH:k * RCH + (c + 1) * RSH, :] = \
                shard[k * RSH:(k + 1) * RSH, :]
    return out.reshape(cfg.B, cfg.L, cfg.M).astype(np.float32)


def kernel(**inputs) -> np.ndarray:
    cfg = FULL
    from concourse.bass_utils import run_bass_kernel_spmd
    nc = build_program(cfg)
    in_maps = host_prep(cfg, inputs)
    res = run_bass_kernel_spmd(nc, in_maps, core_ids=list(range(cfg.n_cores)))
    return gather_out(cfg, res.results)
